# revision 34
# baseline (speedup 1.0000x reference)
"""Multi-head attention + layernorm Bass kernel for Trainium2, 8 cores.

Problem: B=8, S=1024, D=768, H=12 heads x DH=64, key-padding mask, softmax,
output projection, layernorm.  Sharding: pure data parallelism - one batch
element per NeuronCore, no collectives.

v2 design (ACT-exp is the throughput floor at ~110us; everything else must
hide under it):
  - fp8e4(+DoubleRow, K=256/matmul) for q/k/v projections and attn@V; these
    errors enter before the softmax average and wash out.  Scores and the
    output projection stay fp16.
  - weights prescaled x64 on host so fp8 stays in the normal range; the
    scale cancels through the softmax normalize (ctx*64 times 1/(64*den)),
    and for scores it folds into the exp scale 2^-15.
  - iblk-outer attention; out-projection blocks s0..3 interleave under the
    second iblk so only s4..7 are an exposed tail.
  - softmax denominators: ones-column trick in V; reciprocal_approx_fast on
    the psum row; DMA row-broadcast; in-place f16 multiply.
  - ~130 warmup matmuls during the input DMA so HAM reaches K=8/8 before
    real work; xt split across 3 DMA queues.
"""

import numpy as np

B, S, D, H, DH = 8, 1024, 768, 12, 64
NPAIR, NQUAD = H // 2, H // 4
SBLK = S // 128      # 8 key/row chunks
DCH = D // 128       # 6 contraction chunks
LN_EPS = 1e-5
NEG_MASK = -30.0
W64 = 64.0           # host weight prescale
EXP_SCALE = 1.0 / (64.0 * 64.0 * 8.0)   # qt64*kt64 -> scores/8
VW = 65              # per-head stride in V layout: [v64, ones]
VQW = 4 * VW         # 260, per-quad width
FP8 = False
N_WARM = 22

_PROGRAM = None


def _build_program():
    import concourse.bass as bass
    from concourse import bacc
    import concourse.tile as tile
    import concourse.mybir as mybir
    from contextlib import ExitStack

    F32 = mybir.dt.float32
    F16 = mybir.dt.float16
    F8 = mybir.dt.float8e4
    FA = F8 if FP8 else F16
    DR = mybir.MatmulPerfMode.DoubleRow if FP8 else None
    AF = mybir.ActivationFunctionType
    CP = 2 if FP8 else 1          # contraction chunks consumed per matmul

    nc = bacc.Bacc("TRN2", target_bir_lowering=False)

    xt_d = nc.dram_tensor("xt", [128, DCH * S], FA, kind="ExternalInput")
    wq_d = nc.dram_tensor("wq", [NPAIR, 128, DCH * 128], FA, kind="ExternalInput")
    wk_d = nc.dram_tensor("wk", [NPAIR, 128, DCH * 128], FA, kind="ExternalInput")
    wv_d = nc.dram_tensor("wv", [NQUAD, 128, DCH * VQW], FA, kind="ExternalInput")
    wo_d = nc.dram_tensor("wo", [128, DCH * D], F16, kind="ExternalInput")
    bqk_d = nc.dram_tensor("bqk", [128, 2 * NPAIR], F32, kind="ExternalInput")
    bv_d = nc.dram_tensor("bv", [1, NQUAD * VQW], F32, kind="ExternalInput")
    maskb_d = nc.dram_tensor("maskb", [128, SBLK], F32, kind="ExternalInput")
    gamma_d = nc.dram_tensor("gamma", [1, D], F32, kind="ExternalInput")
    beta_d = nc.dram_tensor("beta", [1, D], F32, kind="ExternalInput")
    ones_d = nc.dram_tensor("ones16", [1, 128], F16, kind="ExternalInput")
    onesr_d = nc.dram_tensor("onesr", [1, 128], mybir.dt.float32r,
                             kind="ExternalInput")
    bo_d = nc.dram_tensor("bo16", [1, D], F16, kind="ExternalInput")
    out_d = nc.dram_tensor("out", [S, D], F32, kind="ExternalOutput")

    # j -> (et group, slot in group); groups pair key-chunks for DoubleRow
    ET_SLOT = [(0, 0), (0, 1), (3, 0), (1, 0), (1, 1), (3, 1), (2, 0), (2, 1)]
    # group -> (v dim1 slice start, stop, step)
    GRP_V = {0: (0, 2, 1), 1: (3, 5, 1), 2: (6, 8, 1), 3: (2, 6, 3)}

    with tile.TileContext(nc) as tc, ExitStack() as ctx:
        const = ctx.enter_context(tc.tile_pool(name="const", bufs=1))
        xt_p = ctx.enter_context(tc.tile_pool(name="xt_p", bufs=1))
        w_p = ctx.enter_context(tc.tile_pool(name="w_p", bufs=1))
        qk_p = ctx.enter_context(tc.tile_pool(name="qk_p", bufs=1))
        v_p = ctx.enter_context(tc.tile_pool(name="v_p", bufs=1))
        e_p = ctx.enter_context(tc.tile_pool(name="e_p", bufs=1))
        cx_p = ctx.enter_context(tc.tile_pool(name="cx_p", bufs=1))
        z_p = ctx.enter_context(tc.tile_pool(name="z_p", bufs=1))
        ps = ctx.enter_context(tc.tile_pool(name="ps", bufs=1, space="PSUM"))

        # ---- warmup stationary (DVE memset, no DMA dependency) ----
        warm16 = const.tile([128, 64], F16)
        nc.vector.memset(warm16, 0.25)

        # ---- input DMAs, all on the sync queue: ordered so the first
        # projection (wq0/wk0 + xt) and first exp (bqk, maskb) unblock ASAP
        xt8 = xt_p.tile([128, DCH, S], FA, name="xt8")
        wq_ts = [w_p.tile([128, DCH, 128], FA, name="wqp", bufs=NPAIR)
                 for _ in range(NPAIR)]
        wk_ts = [w_p.tile([128, DCH, 128], FA, name="wkp", bufs=NPAIR)
                 for _ in range(NPAIR)]
        wv_ts = [w_p.tile([128, DCH, VQW], FA, name="wvq", bufs=NQUAD)
                 for _ in range(NQUAD)]
        bqk_t = const.tile([128, 2 * NPAIR], F32)
        mask_t = const.tile([128, SBLK], F32)
        bv_t = const.tile([128, NQUAD * VQW], F32)
        ones_t = const.tile([1, 128], F16)
        onesr_t = const.tile([1, 128], mybir.dt.float32r)
        bo_t = const.tile([1, D], F16)
        gamma_t = const.tile([128, D], F32)
        beta_t = const.tile([128, D], F32)
        woa = w_p.tile([128, DCH, D], F16, name="woa", bufs=1)

        nc.sync.dma_start(out=wq_ts[0], in_=wq_d[0])
        nc.sync.dma_start(out=wk_ts[0], in_=wk_d[0])
        nc.sync.dma_start(out=xt8[:, 0:2, :], in_=xt_d[:, 0:2 * S])
        nc.sync.dma_start(out=xt8[:, 2:4, :], in_=xt_d[:, 2 * S:4 * S])
        nc.sync.dma_start(out=xt8[:, 4:6, :], in_=xt_d[:, 4 * S:6 * S])
        nc.sync.dma_start(out=bqk_t, in_=bqk_d[:, :])
        nc.sync.dma_start(out=mask_t, in_=maskb_d[:, :])
        nc.sync.dma_start(out=wq_ts[1], in_=wq_d[1])
        nc.sync.dma_start(out=wk_ts[1], in_=wk_d[1])
        nc.sync.dma_start(out=ones_t, in_=ones_d[:, :])
        nc.sync.dma_start(out=onesr_t, in_=onesr_d[:, :])
        nc.sync.dma_start(out=bo_t, in_=bo_d[:, :])
        nc.sync.dma_start(out=wq_ts[2], in_=wq_d[2])
        nc.sync.dma_start(out=wk_ts[2], in_=wk_d[2])
        nc.sync.dma_start(out=wv_ts[0], in_=wv_d[0])
        nc.sync.dma_start(out=bv_t, in_=bv_d[0:1, :].to_broadcast([128, NQUAD * VQW]))
        for p in range(3, NPAIR):
            nc.sync.dma_start(out=wq_ts[p], in_=wq_d[p])
            nc.sync.dma_start(out=wk_ts[p], in_=wk_d[p])
        for q in range(1, NQUAD):
            nc.sync.dma_start(out=wv_ts[q], in_=wv_d[q])
        nc.sync.dma_start(out=woa, in_=wo_d[:, :])
        nc.sync.dma_start(out=gamma_t, in_=gamma_d[0:1, :].to_broadcast([128, D]))
        nc.sync.dma_start(out=beta_t, in_=beta_d[0:1, :].to_broadcast([128, D]))
        eps_t = const.tile([128, 1], F32)
        nc.vector.memset(eps_t, LN_EPS)
        magic_t = const.tile([128, 1], mybir.dt.int32)
        nc.vector.memset(magic_t, 0x5F3759DF)

        # ---- PE warmup: keep HAM busy during input DMA ----
        pw = ps.tile([64, 64], F32, name="pw", tag="pa", bufs=2,
                     padded_shape=[128, 512])
        for _ in range(N_WARM):
            nc.tensor.matmul(pw, warm16, warm16, start=True, stop=True)

        # ---- emit helpers ----
        v8 = [v_p.tile([128, SBLK, VQW], FA, name="v8", bufs=NQUAD)
              for _ in range(NQUAD)]
        qt = [qk_p.tile([128, S], F16, name="qt", bufs=NPAIR) for _ in range(NPAIR)]
        kt = [qk_p.tile([128, S], F16, name="kt", bufs=NPAIR) for _ in range(NPAIR)]
        ct = [cx_p.tile([128, S], F16, name="ct", bufs=NPAIR) for _ in range(NPAIR)]

        def emit_vquad(q):
            wv_t = wv_ts[q]
            for s in range(SBLK):
                psv = ps.tile([128, 512], F32, name="psv", tag="pa", bufs=2)
                for ci in range(DCH // CP):
                    nc.tensor.matmul(
                        psv[:, 0:VQW],
                        xt8[:, CP * ci:CP * (ci + 1), s * 128:(s + 1) * 128],
                        wv_t[:, CP * ci:CP * (ci + 1), :],
                        start=(ci == 0), stop=(ci == DCH // CP - 1),
                        perf_mode=DR)
                with tc.high_priority(offset=350):
                    nc.vector.tensor_add(
                        out=v8[q][:, s, :], in0=psv[:, 0:VQW],
                        in1=bv_t[:, q * VQW:(q + 1) * VQW])

        def emit_proj(p):
            for dst, w_t, bcol in ((qt[p], wq_ts[p], p), (kt[p], wk_ts[p], NPAIR + p)):
                for half in range(2):
                    psq = ps.tile([128, 512], F32, name="psq", tag="pa", bufs=2)
                    for ci in range(DCH // CP):
                        nc.tensor.matmul(
                            psq,
                            w_t[:, CP * ci:CP * (ci + 1), :],
                            xt8[:, CP * ci:CP * (ci + 1),
                                half * 512:(half + 1) * 512],
                            start=(ci == 0), stop=(ci == DCH // CP - 1),
                            perf_mode=DR)
                    with tc.high_priority(offset=400):
                        nc.vector.tensor_scalar_add(
                            out=dst[:, half * 512:(half + 1) * 512], in0=psq,
                            scalar1=bqk_t[:, bcol:bcol + 1])

        ET = {}

        def emit_scores(p, iblk):
            # scores + exp only: feeds the ACT engine as early as possible
            ets = []
            for j in range(SBLK):
                pst = ps.tile([128, 1024], F32, name="pst", tag="pb", bufs=2)
                nc.tensor.matmul(
                    pst[:, 0:512], kt[p][0:64, j * 128:(j + 1) * 128],
                    qt[p][0:64, iblk * 512:(iblk + 1) * 512],
                    start=True, stop=True, tile_position=(0, 0))
                nc.tensor.matmul(
                    pst[:, 512:1024], kt[p][64:128, j * 128:(j + 1) * 128],
                    qt[p][64:128, iblk * 512:(iblk + 1) * 512],
                    start=True, stop=True, tile_position=(64, 0))
                et = e_p.tile([128, 1024], FA, name="et", bufs=24)
                nc.scalar.activation(et, pst, AF.Exp,
                                     bias=mask_t[:, j:j + 1],
                                     scale=EXP_SCALE)
                ets.append(et)
            ET[(p, iblk)] = ets

        def emit_ctx(p, iblk):
            # emitted >=9us of PE work after emit_scores so the exps are done
            # by the time the PE stream reaches these matmuls (in-order PE)
            qx = 2 * p // 4
            l0 = (2 * p) % 4
            ets = ET.pop((p, iblk))
            pcx = ps.tile([65, 1024], F32, name="pcx", tag="pc", bufs=1)
            for j in range(SBLK):
                for idx in range(2):
                    nc.tensor.matmul(
                        pcx[0:65, idx * 512:(idx + 1) * 512],
                        v8[qx][:, j, (l0 + idx) * VW:(l0 + idx + 1) * VW],
                        ets[j][:, idx * 512:(idx + 1) * 512],
                        start=(j == 0), stop=(j == SBLK - 1))
            # denominator row out first (releases nothing yet but starts
            # the chain); then normalize happens IN the psum->ct move
            rxs = z_p.tile([1, 1024], F32, name="rxs", bufs=3)
            rx = z_p.tile([1, 1024], F32, name="rx", bufs=3)
            with tc.high_priority(offset=300):
                nc.vector.tensor_copy(out=rxs, in_=pcx[64:65, 0:1024])
            nc.vector.reciprocal_approx_fast(out=rx, in_=rxs)
            rx16 = z_p.tile([1, 1024], F16, name="rx16", bufs=3)
            nc.vector.tensor_copy(out=rx16, in_=rx)
            # broadcast 1/den across partitions via K=1 f16 matmuls (PE),
            # cast to sbuf f16, then a single fused multiply writes ct
            pbc = ps.tile([128, 512], F32, name="pbc", tag="pa", bufs=2)
            nc.tensor.matmul(pbc[0:64, :], ones_t[0:1, 0:64],
                             rx16[0:1, 0:512], start=True, stop=True)
            nc.tensor.matmul(pbc[64:128, :], ones_t[0:1, 0:64],
                             rx16[0:1, 512:1024], start=True, stop=True,
                             tile_position=(0, 64))
            pb16 = z_p.tile([128, 512], F16, name="pb16", bufs=3)
            nc.vector.tensor_copy(out=pb16, in_=pbc)
            nc.vector.tensor_mul(
                out=ct[p][0:64, iblk * 512:(iblk + 1) * 512],
                in0=pcx[0:64, 0:512], in1=pb16[0:64, :])
            nc.vector.tensor_mul(
                out=ct[p][64:128, iblk * 512:(iblk + 1) * 512],
                in0=pcx[0:64, 512:1024], in1=pb16[64:128, :])

        def emit_out(s):
            # alternate psum rings so consecutive out-blocks never wait on
            # each other's LN drain (depth-2 pipeline in the tail)
            ring = "pa" if s % 2 == 0 else "pb"
            pso_a = ps.tile([128, 512], F32, name="pso_a", tag=ring, bufs=2)
            pso_b = ps.tile([128, 512], F32, name="pso_b", tag=ring, bufs=2)
            for pt, d0, dn in ((pso_a, 0, 512), (pso_b, 512, 256)):
                for c in range(NPAIR):
                    nc.tensor.matmul(
                        pt[:, 0:dn],
                        ct[c][:, s * 128:(s + 1) * 128],
                        woa[:, c, d0:d0 + dn],
                        start=(c == 0), stop=False)
                nc.tensor.matmul(pt[:, 0:dn], ones_t,
                                 bo_t[0:1, d0:d0 + dn],
                                 start=False, stop=True)
            stats = z_p.tile([128, 3, 6], F32, name="stats", bufs=2)
            with tc.high_priority(offset=600):
                nc.vector.bn_stats(out=stats[:, 0, :], in_=pso_a[:, 0:256])
                nc.vector.bn_stats(out=stats[:, 1, :], in_=pso_a[:, 256:512])
                nc.vector.bn_stats(out=stats[:, 2, :], in_=pso_b[:, 0:256])
                mv = z_p.tile([128, 2], F32, name="mv", bufs=2)
                nc.vector.bn_aggr(out=mv, in_=stats)
            # rstd = rsqrt(var+eps) via quake seed + 2 Newton steps, all on
            # DVE: keeps the ACT engine exp-only (no table-set thrash)
            I32 = mybir.dt.int32
            with tc.high_priority(offset=600):
                veps = z_p.tile([128, 1], F32, name="veps", bufs=2)
                nc.vector.tensor_scalar_add(out=veps, in0=mv[:, 1:2],
                                            scalar1=LN_EPS)
                hb = z_p.tile([128, 1], I32, name="hb", bufs=2)
                nc.vector.tensor_scalar(out=hb, in0=veps.bitcast(I32),
                                        scalar1=1, scalar2=None,
                                        op0=mybir.AluOpType.arith_shift_right)
                y0 = z_p.tile([128, 1], I32, name="y0", bufs=2)
                nc.vector.tensor_tensor(out=y0, in0=magic_t, in1=hb,
                                        op=mybir.AluOpType.subtract)
                rstd = y0.bitcast(F32)
                vm = z_p.tile([128, 1], F32, name="vm", bufs=2)
                nc.vector.tensor_scalar_mul(out=vm, in0=veps, scalar1=-0.5)
                tq = z_p.tile([128, 1], F32, name="tq", bufs=2)
                for _ in range(2):
                    # 2 Newton steps: rstd rel err ~5e-6
                    nc.vector.tensor_mul(out=tq, in0=rstd, in1=rstd)
                    nc.vector.tensor_scalar(out=tq, in0=tq, scalar1=vm,
                                            scalar2=1.5,
                                            op0=mybir.AluOpType.mult,
                                            op1=mybir.AluOpType.add)
                    nc.vector.tensor_mul(out=y0.bitcast(F32), in0=rstd, in1=tq)
                nmr = z_p.tile([128, 1], F32, name="nmr", bufs=2)
                nc.vector.tensor_scalar(out=nmr, in0=mv[:, 0:1], scalar1=rstd,
                                        scalar2=-1.0, op0=mybir.AluOpType.mult,
                                        op1=mybir.AluOpType.mult)
                z = z_p.tile([128, D], F32, name="z_sb", bufs=2)
                if s >= 4:
                    # tail: ACT is idle once the exps are done
                    nc.scalar.activation(z[:, 0:512], pso_a, AF.Identity,
                                         bias=nmr, scale=rstd)
                    nc.scalar.activation(z[:, 512:768], pso_b[:, 0:256],
                                         AF.Identity, bias=nmr, scale=rstd)
                else:
                    nc.vector.tensor_scalar(out=z[:, 0:512], in0=pso_a,
                                            scalar1=rstd, scalar2=nmr,
                                            op0=mybir.AluOpType.mult,
                                            op1=mybir.AluOpType.add)
                    nc.vector.tensor_scalar(out=z[:, 512:768],
                                            in0=pso_b[:, 0:256],
                                            scalar1=rstd, scalar2=nmr,
                                            op0=mybir.AluOpType.mult,
                                            op1=mybir.AluOpType.add)
            nc.gpsimd.tensor_mul(out=z, in0=z, in1=gamma_t)
            zf = z_p.tile([128, D], F32, name="zf", bufs=2)
            nc.gpsimd.tensor_add(out=zf, in0=z, in1=beta_t)
            nc.sync.dma_start(out=out_d[s * 128:(s + 1) * 128, :], in_=zf)

        # ---- emission schedule: scores early (feed ACT), ctx late enough
        # that its exps are complete when the in-order PE reaches it ----
        emit_proj(0)
        emit_scores(0, 0)
        emit_proj(1)
        emit_scores(1, 0)
        emit_vquad(0)
        emit_ctx(0, 0)
        emit_proj(2)
        emit_scores(2, 0)
        emit_ctx(1, 0)
        emit_vquad(1)
        emit_proj(3)
        emit_scores(3, 0)
        emit_ctx(2, 0)
        emit_proj(4)
        emit_scores(4, 0)
        emit_vquad(2)
        emit_ctx(3, 0)
        emit_proj(5)
        emit_scores(5, 0)
        emit_scores(0, 1)
        emit_ctx(4, 0)
        emit_scores(1, 1)
        emit_ctx(5, 0)
        emit_ctx(0, 1)
        emit_out(0)
        emit_scores(2, 1)
        emit_ctx(1, 1)
        emit_out(1)
        emit_scores(3, 1)
        emit_ctx(2, 1)
        emit_out(2)
        emit_scores(4, 1)
        emit_ctx(3, 1)
        emit_out(3)
        emit_scores(5, 1)
        emit_ctx(4, 1)
        emit_ctx(5, 1)
        for s in range(4, SBLK):
            emit_out(s)

    nc.compile()
    return nc


def _np_f8():
    import ml_dtypes
    return ml_dtypes.float8_e4m3fn


def _host_inputs(inputs):
    x = np.asarray(inputs["input_tensor"], np.float32)
    mask = np.asarray(inputs["attention_mask"])
    Wq = np.asarray(inputs["Wq"], np.float32)
    bq = np.asarray(inputs["bq"], np.float32)
    Wk = np.asarray(inputs["Wk"], np.float32)
    bk = np.asarray(inputs["bk"], np.float32)
    Wv = np.asarray(inputs["Wv"], np.float32)
    bv = np.asarray(inputs["bv"], np.float32)
    Wo = np.asarray(inputs["Wo"], np.float32)
    bo = np.asarray(inputs["bo"], np.float32)
    gamma = np.asarray(inputs["gamma"], np.float32)
    beta = np.asarray(inputs["beta"], np.float32)

    fa = _np_f8() if FP8 else np.float16

    wq_flat = np.ascontiguousarray(Wq.transpose(1, 0, 2).reshape(D, D)) * W64
    wk_flat = np.ascontiguousarray(Wk.transpose(1, 0, 2).reshape(D, D)) * W64
    bq_s = bq.reshape(D) * W64
    bk_s = bk.reshape(D) * W64

    # ones column FIRST per head: denominator lands at psum partition 0
    wv_aug = np.zeros((D, NQUAD * VQW), np.float32)
    bv_aug = np.zeros((1, NQUAD * VQW), np.float32)
    for h in range(H):
        q, l = divmod(h, 4)
        base = q * VQW + l * VW
        wv_aug[:, base:base + 64] = Wv[h] * W64
        bv_aug[0, base:base + 64] = bv[h] * W64
        bv_aug[0, base + 64] = W64

    bqk = np.zeros((128, 2 * NPAIR), np.float32)
    for p in range(NPAIR):
        bqk[:, p] = bq_s[p * 128:(p + 1) * 128]
        bqk[:, NPAIR + p] = bk_s[p * 128:(p + 1) * 128]

    def sbuf_layout(w, width, dt):
        # [D, n*width] -> [n, 128, DCH*width]: partition-major per tile
        n = w.shape[1] // width
        return np.ascontiguousarray(
            w.reshape(DCH, 128, n, width).transpose(2, 1, 0, 3).reshape(
                n, 128, DCH * width).astype(dt))

    shared = {
        "wq": sbuf_layout(wq_flat, 128, fa),
        "wk": sbuf_layout(wk_flat, 128, fa),
        "wv": sbuf_layout(wv_aug, VQW, fa),
        "wo": sbuf_layout(np.ascontiguousarray(Wo), D, np.float16)[0],
        "bqk": bqk, "bv": bv_aug,
        "gamma": gamma.reshape(1, D).copy(),
        "beta": beta.reshape(1, D).copy(),
        "ones16": np.ones((1, 128), np.float16),
        "onesr": np.ones((1, 128), np.float32),
        "bo16": bo.reshape(1, D).astype(np.float16),
    }
    in_maps = []
    for b in range(B):
        mb = np.where(mask[b], 0.0, NEG_MASK).astype(np.float32)
        in_maps.append({
            **shared,
            "xt": np.ascontiguousarray(
                x[b].T.reshape(DCH, 128, S).transpose(1, 0, 2).reshape(
                    128, DCH * S).astype(fa)),
            "maskb": np.ascontiguousarray(mb.reshape(SBLK, 128).T),
        })
    return in_maps


def _get_program():
    global _PROGRAM
    if _PROGRAM is None:
        _PROGRAM = _build_program()
    return _PROGRAM


def kernel(**inputs):
    from concourse.bass_utils import run_bass_kernel_spmd

    nc = _get_program()
    in_maps = _host_inputs(inputs)
    res = run_bass_kernel_spmd(nc, in_maps, list(range(B)))
    return np.stack([res.results[b]["out"] for b in range(B)], axis=0)


if __name__ == "__main__":
    rng = np.random.default_rng(0)
    demo = {
        "input_tensor": rng.standard_normal((B, S, D)).astype(np.float32),
        "attention_mask": np.ones((B, S), bool),
        "Wq": (rng.standard_normal((H, D, DH)) * 0.03).astype(np.float32),
        "bq": (rng.standard_normal((H, DH)) * 0.03).astype(np.float32),
        "Wk": (rng.standard_normal((H, D, DH)) * 0.03).astype(np.float32),
        "bk": (rng.standard_normal((H, DH)) * 0.03).astype(np.float32),
        "Wv": (rng.standard_normal((H, D, DH)) * 0.03).astype(np.float32),
        "bv": (rng.standard_normal((H, DH)) * 0.03).astype(np.float32),
        "Wo": (rng.standard_normal((D, D)) * 0.03).astype(np.float32),
        "bo": (rng.standard_normal((D,)) * 0.03).astype(np.float32),
        "gamma": np.ones((D,), np.float32),
        "beta": np.zeros((D,), np.float32),
    }
    out = kernel(**demo)
    print("kernel ran, out shape", out.shape, "finite:", np.isfinite(out).all())


# revision 37
# speedup vs baseline: 1.0115x; 1.0115x over previous
"""Multi-head attention + layernorm Bass kernel for Trainium2, 8 cores.

Problem: B=8, S=1024, D=768, H=12 heads x DH=64, key-padding mask, softmax,
output projection, layernorm.  Sharding: pure data parallelism - one batch
element per NeuronCore, no collectives.

v2 design (ACT-exp is the throughput floor at ~110us; everything else must
hide under it):
  - fp8e4(+DoubleRow, K=256/matmul) for q/k/v projections and attn@V; these
    errors enter before the softmax average and wash out.  Scores and the
    output projection stay fp16.
  - weights prescaled x64 on host so fp8 stays in the normal range; the
    scale cancels through the softmax normalize (ctx*64 times 1/(64*den)),
    and for scores it folds into the exp scale 2^-15.
  - iblk-outer attention; out-projection blocks s0..3 interleave under the
    second iblk so only s4..7 are an exposed tail.
  - softmax denominators: ones-column trick in V; reciprocal_approx_fast on
    the psum row; DMA row-broadcast; in-place f16 multiply.
  - ~130 warmup matmuls during the input DMA so HAM reaches K=8/8 before
    real work; xt split across 3 DMA queues.
"""

import numpy as np

B, S, D, H, DH = 8, 1024, 768, 12, 64
NPAIR, NQUAD = H // 2, H // 4
SBLK = S // 128      # 8 key/row chunks
DCH = D // 128       # 6 contraction chunks
LN_EPS = 1e-5
NEG_MASK = -30.0
W64 = 64.0           # host weight prescale
EXP_SCALE = 1.0 / (64.0 * 64.0 * 8.0)   # qt64*kt64 -> scores/8
VW = 65              # per-head stride in V layout: [v64, ones]
VQW = 4 * VW         # 260, per-quad width
FP8 = False
N_WARM = 22

_PROGRAM = None


def _build_program():
    import concourse.bass as bass
    from concourse import bacc
    import concourse.tile as tile
    import concourse.mybir as mybir
    from contextlib import ExitStack

    F32 = mybir.dt.float32
    F16 = mybir.dt.float16
    F8 = mybir.dt.float8e4
    FA = F8 if FP8 else F16
    DR = mybir.MatmulPerfMode.DoubleRow if FP8 else None
    AF = mybir.ActivationFunctionType
    CP = 2 if FP8 else 1          # contraction chunks consumed per matmul

    nc = bacc.Bacc("TRN2", target_bir_lowering=False)

    xt_d = nc.dram_tensor("xt", [128, DCH * S], FA, kind="ExternalInput")
    wq_d = nc.dram_tensor("wq", [NPAIR, 128, DCH * 128], FA, kind="ExternalInput")
    wk_d = nc.dram_tensor("wk", [NPAIR, 128, DCH * 128], FA, kind="ExternalInput")
    wv_d = nc.dram_tensor("wv", [NQUAD, 128, DCH * VQW], FA, kind="ExternalInput")
    wo_d = nc.dram_tensor("wo", [128, DCH * D], F16, kind="ExternalInput")
    bqk_d = nc.dram_tensor("bqk", [128, 2 * NPAIR], F32, kind="ExternalInput")
    bv_d = nc.dram_tensor("bv", [1, NQUAD * VQW], F32, kind="ExternalInput")
    maskb_d = nc.dram_tensor("maskb", [128, SBLK], F32, kind="ExternalInput")
    gamma_d = nc.dram_tensor("gamma", [1, D], F32, kind="ExternalInput")
    beta_d = nc.dram_tensor("beta", [1, D], F32, kind="ExternalInput")
    ones_d = nc.dram_tensor("ones16", [1, 128], F16, kind="ExternalInput")
    onesr_d = nc.dram_tensor("onesr", [1, 128], mybir.dt.float32r,
                             kind="ExternalInput")
    bo_d = nc.dram_tensor("bo16", [1, D], F16, kind="ExternalInput")
    out_d = nc.dram_tensor("out", [S, D], F32, kind="ExternalOutput")

    # j -> (et group, slot in group); groups pair key-chunks for DoubleRow
    ET_SLOT = [(0, 0), (0, 1), (3, 0), (1, 0), (1, 1), (3, 1), (2, 0), (2, 1)]
    # group -> (v dim1 slice start, stop, step)
    GRP_V = {0: (0, 2, 1), 1: (3, 5, 1), 2: (6, 8, 1), 3: (2, 6, 3)}

    with tile.TileContext(nc) as tc, ExitStack() as ctx:
        const = ctx.enter_context(tc.tile_pool(name="const", bufs=1))
        xt_p = ctx.enter_context(tc.tile_pool(name="xt_p", bufs=1))
        w_p = ctx.enter_context(tc.tile_pool(name="w_p", bufs=1))
        qk_p = ctx.enter_context(tc.tile_pool(name="qk_p", bufs=1))
        v_p = ctx.enter_context(tc.tile_pool(name="v_p", bufs=1))
        e_p = ctx.enter_context(tc.tile_pool(name="e_p", bufs=1))
        cx_p = ctx.enter_context(tc.tile_pool(name="cx_p", bufs=1))
        z_p = ctx.enter_context(tc.tile_pool(name="z_p", bufs=1))
        ps = ctx.enter_context(tc.tile_pool(name="ps", bufs=1, space="PSUM"))

        # ---- warmup stationary (DVE memset, no DMA dependency) ----
        warm16 = const.tile([128, 64], F16)
        nc.vector.memset(warm16, 0.25)

        # ---- input DMAs, all on the sync queue: ordered so the first
        # projection (wq0/wk0 + xt) and first exp (bqk, maskb) unblock ASAP
        xt8 = xt_p.tile([128, DCH, S], FA, name="xt8")
        wq_ts = [w_p.tile([128, DCH, 128], FA, name="wqp", bufs=NPAIR)
                 for _ in range(NPAIR)]
        wk_ts = [w_p.tile([128, DCH, 128], FA, name="wkp", bufs=NPAIR)
                 for _ in range(NPAIR)]
        wv_ts = [w_p.tile([128, DCH, VQW], FA, name="wvq", bufs=NQUAD)
                 for _ in range(NQUAD)]
        bqk_t = const.tile([128, 2 * NPAIR], F32)
        mask_t = const.tile([128, SBLK], F32)
        bv_t = const.tile([128, NQUAD * VQW], F32)
        ones_t = const.tile([1, 128], F16)
        onesr_t = const.tile([1, 128], mybir.dt.float32r)
        bo_t = const.tile([1, D], F16)
        gamma_t = const.tile([128, D], F32)
        beta_t = const.tile([128, D], F32)
        woa = w_p.tile([128, DCH, D], F16, name="woa", bufs=1)

        nc.sync.dma_start(out=wq_ts[0], in_=wq_d[0])
        nc.sync.dma_start(out=wk_ts[0], in_=wk_d[0])
        nc.sync.dma_start(out=xt8[:, 0:2, :], in_=xt_d[:, 0:2 * S])
        nc.sync.dma_start(out=xt8[:, 2:4, :], in_=xt_d[:, 2 * S:4 * S])
        nc.sync.dma_start(out=xt8[:, 4:6, :], in_=xt_d[:, 4 * S:6 * S])
        nc.sync.dma_start(out=bqk_t, in_=bqk_d[:, :])
        nc.sync.dma_start(out=mask_t, in_=maskb_d[:, :])
        nc.sync.dma_start(out=wq_ts[1], in_=wq_d[1])
        nc.sync.dma_start(out=wk_ts[1], in_=wk_d[1])
        nc.sync.dma_start(out=ones_t, in_=ones_d[:, :])
        nc.sync.dma_start(out=onesr_t, in_=onesr_d[:, :])
        nc.sync.dma_start(out=bo_t, in_=bo_d[:, :])
        nc.sync.dma_start(out=wq_ts[2], in_=wq_d[2])
        nc.sync.dma_start(out=wk_ts[2], in_=wk_d[2])
        nc.sync.dma_start(out=wv_ts[0], in_=wv_d[0])
        nc.sync.dma_start(out=bv_t, in_=bv_d[0:1, :].to_broadcast([128, NQUAD * VQW]))
        for p in range(3, NPAIR):
            nc.sync.dma_start(out=wq_ts[p], in_=wq_d[p])
            nc.sync.dma_start(out=wk_ts[p], in_=wk_d[p])
        for q in range(1, NQUAD):
            nc.sync.dma_start(out=wv_ts[q], in_=wv_d[q])
        nc.sync.dma_start(out=woa, in_=wo_d[:, :])
        nc.sync.dma_start(out=gamma_t, in_=gamma_d[0:1, :].to_broadcast([128, D]))
        nc.sync.dma_start(out=beta_t, in_=beta_d[0:1, :].to_broadcast([128, D]))
        eps_t = const.tile([128, 1], F32)
        nc.vector.memset(eps_t, LN_EPS)
        magic_t = const.tile([128, 1], mybir.dt.int32)
        nc.vector.memset(magic_t, 0x5F3759DF)

        # ---- PE warmup: keep HAM busy during input DMA ----
        pw = ps.tile([64, 64], F32, name="pw", tag="pa", bufs=2,
                     padded_shape=[128, 512])
        for _ in range(N_WARM):
            nc.tensor.matmul(pw, warm16, warm16, start=True, stop=True)

        # ---- emit helpers ----
        v8 = [v_p.tile([128, SBLK, VQW], FA, name="v8", bufs=NQUAD)
              for _ in range(NQUAD)]
        qt = [qk_p.tile([128, S], F16, name="qt", bufs=NPAIR) for _ in range(NPAIR)]
        kt = [qk_p.tile([128, S], F16, name="kt", bufs=NPAIR) for _ in range(NPAIR)]
        ct = [cx_p.tile([128, S], F16, name="ct", bufs=NPAIR) for _ in range(NPAIR)]

        def emit_vquad(q):
            wv_t = wv_ts[q]

            def mk_v(s):
                def f():
                    psv = ps.tile([128, 512], F32, name="psv", tag="pa",
                                  bufs=2)
                    for ci in range(DCH // CP):
                        nc.tensor.matmul(
                            psv[:, 0:VQW],
                            xt8[:, CP * ci:CP * (ci + 1),
                                s * 128:(s + 1) * 128],
                            wv_t[:, CP * ci:CP * (ci + 1), :],
                            start=(ci == 0), stop=(ci == DCH // CP - 1),
                            perf_mode=DR)
                    with tc.high_priority(offset=350):
                        nc.vector.tensor_add(
                            out=v8[q][:, s, :], in0=psv[:, 0:VQW],
                            in1=bv_t[:, q * VQW:(q + 1) * VQW])
                return f

            for s in range(SBLK):
                pending.append(mk_v(s))

        def emit_proj(p):
            for dst, w_t, bcol in ((qt[p], wq_ts[p], p), (kt[p], wk_ts[p], NPAIR + p)):
                for half in range(2):
                    psq = ps.tile([128, 512], F32, name="psq", tag="pa", bufs=2)
                    for ci in range(DCH // CP):
                        nc.tensor.matmul(
                            psq,
                            w_t[:, CP * ci:CP * (ci + 1), :],
                            xt8[:, CP * ci:CP * (ci + 1),
                                half * 512:(half + 1) * 512],
                            start=(ci == 0), stop=(ci == DCH // CP - 1),
                            perf_mode=DR)
                    with tc.high_priority(offset=400):
                        nc.vector.tensor_scalar_add(
                            out=dst[:, half * 512:(half + 1) * 512], in0=psq,
                            scalar1=bqk_t[:, bcol:bcol + 1])

        ET = {}
        from collections import deque
        pending = deque()

        def fill(n):
            for _ in range(n):
                if pending:
                    pending.popleft()()

        def flush():
            while pending:
                pending.popleft()()

        def emit_scores(p, iblk):
            # scores + exp feed ACT; after each slot, drain two queued
            # dependency-free PE work units so the in-order PE never idles
            ets = []
            for j in range(SBLK):
                pst = ps.tile([128, 1024], F32, name="pst", tag="pb", bufs=2)
                nc.tensor.matmul(
                    pst[:, 0:512], kt[p][0:64, j * 128:(j + 1) * 128],
                    qt[p][0:64, iblk * 512:(iblk + 1) * 512],
                    start=True, stop=True, tile_position=(0, 0))
                nc.tensor.matmul(
                    pst[:, 512:1024], kt[p][64:128, j * 128:(j + 1) * 128],
                    qt[p][64:128, iblk * 512:(iblk + 1) * 512],
                    start=True, stop=True, tile_position=(64, 0))
                et = e_p.tile([128, 1024], FA, name="et", bufs=24)
                nc.scalar.activation(et, pst, AF.Exp,
                                     bias=mask_t[:, j:j + 1],
                                     scale=EXP_SCALE)
                ets.append(et)
                fill(2)
            ET[(p, iblk)] = ets

        def emit_ctx(p, iblk):
            qx = 2 * p // 4
            l0 = (2 * p) % 4
            ets = ET.pop((p, iblk))
            box = []

            def mk_ctx(j):
                def f():
                    if j == 0:
                        box.append(ps.tile([65, 1024], F32, name="pcx",
                                           tag="pc", bufs=1))
                    pcx = box[0]
                    for idx in range(2):
                        nc.tensor.matmul(
                            pcx[0:65, idx * 512:(idx + 1) * 512],
                            v8[qx][:, j,
                                   (l0 + idx) * VW:(l0 + idx + 1) * VW],
                            ets[j][:, idx * 512:(idx + 1) * 512],
                            start=(j == 0), stop=(j == SBLK - 1))
                return f

            def norm():
                pcx = box[0]
                rxs = z_p.tile([1, 1024], F32, name="rxs", bufs=3)
                rx = z_p.tile([1, 1024], F32, name="rx", bufs=3)
                with tc.high_priority(offset=300):
                    nc.vector.tensor_copy(out=rxs, in_=pcx[64:65, 0:1024])
                nc.vector.reciprocal_approx_fast(out=rx, in_=rxs)
                rx16 = z_p.tile([1, 1024], F16, name="rx16", bufs=3)
                nc.vector.tensor_copy(out=rx16, in_=rx)
                pbc = ps.tile([128, 512], F32, name="pbc", tag="pa", bufs=2)
                nc.tensor.matmul(pbc[0:64, :], ones_t[0:1, 0:64],
                                 rx16[0:1, 0:512], start=True, stop=True)
                nc.tensor.matmul(pbc[64:128, :], ones_t[0:1, 0:64],
                                 rx16[0:1, 512:1024], start=True, stop=True,
                                 tile_position=(0, 64))
                pb16 = z_p.tile([128, 512], F16, name="pb16", bufs=3)
                nc.vector.tensor_copy(out=pb16, in_=pbc)
                nc.vector.tensor_mul(
                    out=ct[p][0:64, iblk * 512:(iblk + 1) * 512],
                    in0=pcx[0:64, 0:512], in1=pb16[0:64, :])
                nc.vector.tensor_mul(
                    out=ct[p][64:128, iblk * 512:(iblk + 1) * 512],
                    in0=pcx[0:64, 512:1024], in1=pb16[64:128, :])

            for j in range(SBLK):
                pending.append(mk_ctx(j))
            pending.append(norm)

        def emit_out(s, direct=True):
            # alternate psum rings so consecutive out-blocks never wait on
            # each other's LN drain (depth-2 pipeline in the tail)
            ring = "pa" if s % 2 == 0 else "pb"
            box = {}

            def mk_half(key, d0, dn):
                def f():
                    pt = ps.tile([128, 512], F32, name="pso_" + key,
                                 tag=ring, bufs=2)
                    box[key] = pt
                    for c in range(NPAIR):
                        nc.tensor.matmul(
                            pt[:, 0:dn],
                            ct[c][:, s * 128:(s + 1) * 128],
                            woa[:, c, d0:d0 + dn],
                            start=(c == 0), stop=False)
                    nc.tensor.matmul(pt[:, 0:dn], ones_t,
                                     bo_t[0:1, d0:d0 + dn],
                                     start=False, stop=True)
                return f

            def ln():
                emit_ln(s, box["a"], box["b"])

            units = [mk_half("a", 0, 512), mk_half("b", 512, 256), ln]
            if direct:
                for u in units:
                    u()
            else:
                pending.extend(units)

        def emit_ln(s, pso_a, pso_b):
            stats = z_p.tile([128, 3, 6], F32, name="stats", bufs=2)
            with tc.high_priority(offset=600):
                nc.vector.bn_stats(out=stats[:, 0, :], in_=pso_a[:, 0:256])
                nc.vector.bn_stats(out=stats[:, 1, :], in_=pso_a[:, 256:512])
                nc.vector.bn_stats(out=stats[:, 2, :], in_=pso_b[:, 0:256])
                mv = z_p.tile([128, 2], F32, name="mv", bufs=2)
                nc.vector.bn_aggr(out=mv, in_=stats)
            # rstd = rsqrt(var+eps) via quake seed + 2 Newton steps, all on
            # DVE: keeps the ACT engine exp-only (no table-set thrash)
            I32 = mybir.dt.int32
            with tc.high_priority(offset=600):
                veps = z_p.tile([128, 1], F32, name="veps", bufs=2)
                nc.vector.tensor_scalar_add(out=veps, in0=mv[:, 1:2],
                                            scalar1=LN_EPS)
                hb = z_p.tile([128, 1], I32, name="hb", bufs=2)
                nc.vector.tensor_scalar(out=hb, in0=veps.bitcast(I32),
                                        scalar1=1, scalar2=None,
                                        op0=mybir.AluOpType.arith_shift_right)
                y0 = z_p.tile([128, 1], I32, name="y0", bufs=2)
                nc.vector.tensor_tensor(out=y0, in0=magic_t, in1=hb,
                                        op=mybir.AluOpType.subtract)
                rstd = y0.bitcast(F32)
                vm = z_p.tile([128, 1], F32, name="vm", bufs=2)
                nc.vector.tensor_scalar_mul(out=vm, in0=veps, scalar1=-0.5)
                tq = z_p.tile([128, 1], F32, name="tq", bufs=2)
                for _ in range(2):
                    # 2 Newton steps: rstd rel err ~5e-6
                    nc.vector.tensor_mul(out=tq, in0=rstd, in1=rstd)
                    nc.vector.tensor_scalar(out=tq, in0=tq, scalar1=vm,
                                            scalar2=1.5,
                                            op0=mybir.AluOpType.mult,
                                            op1=mybir.AluOpType.add)
                    nc.vector.tensor_mul(out=y0.bitcast(F32), in0=rstd, in1=tq)
                nmr = z_p.tile([128, 1], F32, name="nmr", bufs=2)
                nc.vector.tensor_scalar(out=nmr, in0=mv[:, 0:1], scalar1=rstd,
                                        scalar2=-1.0, op0=mybir.AluOpType.mult,
                                        op1=mybir.AluOpType.mult)
                z = z_p.tile([128, D], F32, name="z_sb", bufs=2)
                if s >= 4:
                    # tail: ACT is idle once the exps are done
                    nc.scalar.activation(z[:, 0:512], pso_a, AF.Identity,
                                         bias=nmr, scale=rstd)
                    nc.scalar.activation(z[:, 512:768], pso_b[:, 0:256],
                                         AF.Identity, bias=nmr, scale=rstd)
                else:
                    nc.vector.tensor_scalar(out=z[:, 0:512], in0=pso_a,
                                            scalar1=rstd, scalar2=nmr,
                                            op0=mybir.AluOpType.mult,
                                            op1=mybir.AluOpType.add)
                    nc.vector.tensor_scalar(out=z[:, 512:768],
                                            in0=pso_b[:, 0:256],
                                            scalar1=rstd, scalar2=nmr,
                                            op0=mybir.AluOpType.mult,
                                            op1=mybir.AluOpType.add)
            nc.gpsimd.tensor_mul(out=z, in0=z, in1=gamma_t)
            zf = z_p.tile([128, D], F32, name="zf", bufs=2)
            nc.gpsimd.tensor_add(out=zf, in0=z, in1=beta_t)
            nc.sync.dma_start(out=out_d[s * 128:(s + 1) * 128, :], in_=zf)

        # ---- braided emission: scores slots drain queued dep-free PE
        # units (ctx of earlier pairs, V blocks, out blocks) so the
        # in-order PE stream never idles on exp-paced dependencies ----
        emit_proj(0)
        emit_vquad(0)
        emit_scores(0, 0)
        emit_proj(1)
        emit_ctx(0, 0)
        emit_scores(1, 0)
        emit_proj(2)
        emit_vquad(1)
        emit_scores(2, 0)
        emit_proj(3)
        emit_ctx(1, 0)
        emit_scores(3, 0)
        emit_proj(4)
        emit_vquad(2)
        emit_ctx(2, 0)
        emit_scores(4, 0)
        emit_proj(5)
        emit_ctx(3, 0)
        emit_scores(5, 0)
        emit_ctx(4, 0)
        emit_scores(0, 1)
        emit_ctx(5, 0)
        emit_scores(1, 1)
        emit_ctx(0, 1)
        emit_scores(2, 1)
        emit_ctx(1, 1)
        emit_out(0, direct=False)
        emit_scores(3, 1)
        emit_ctx(2, 1)
        emit_out(1, direct=False)
        emit_scores(4, 1)
        emit_ctx(3, 1)
        emit_out(2, direct=False)
        emit_scores(5, 1)
        emit_out(3, direct=False)
        emit_ctx(4, 1)
        emit_ctx(5, 1)
        flush()
        for s in range(4, SBLK):
            emit_out(s)

    nc.compile()
    return nc


def _np_f8():
    import ml_dtypes
    return ml_dtypes.float8_e4m3fn


def _host_inputs(inputs):
    x = np.asarray(inputs["input_tensor"], np.float32)
    mask = np.asarray(inputs["attention_mask"])
    Wq = np.asarray(inputs["Wq"], np.float32)
    bq = np.asarray(inputs["bq"], np.float32)
    Wk = np.asarray(inputs["Wk"], np.float32)
    bk = np.asarray(inputs["bk"], np.float32)
    Wv = np.asarray(inputs["Wv"], np.float32)
    bv = np.asarray(inputs["bv"], np.float32)
    Wo = np.asarray(inputs["Wo"], np.float32)
    bo = np.asarray(inputs["bo"], np.float32)
    gamma = np.asarray(inputs["gamma"], np.float32)
    beta = np.asarray(inputs["beta"], np.float32)

    fa = _np_f8() if FP8 else np.float16

    wq_flat = np.ascontiguousarray(Wq.transpose(1, 0, 2).reshape(D, D)) * W64
    wk_flat = np.ascontiguousarray(Wk.transpose(1, 0, 2).reshape(D, D)) * W64
    bq_s = bq.reshape(D) * W64
    bk_s = bk.reshape(D) * W64

    # ones column FIRST per head: denominator lands at psum partition 0
    wv_aug = np.zeros((D, NQUAD * VQW), np.float32)
    bv_aug = np.zeros((1, NQUAD * VQW), np.float32)
    for h in range(H):
        q, l = divmod(h, 4)
        base = q * VQW + l * VW
        wv_aug[:, base:base + 64] = Wv[h] * W64
        bv_aug[0, base:base + 64] = bv[h] * W64
        bv_aug[0, base + 64] = W64

    bqk = np.zeros((128, 2 * NPAIR), np.float32)
    for p in range(NPAIR):
        bqk[:, p] = bq_s[p * 128:(p + 1) * 128]
        bqk[:, NPAIR + p] = bk_s[p * 128:(p + 1) * 128]

    def sbuf_layout(w, width, dt):
        # [D, n*width] -> [n, 128, DCH*width]: partition-major per tile
        n = w.shape[1] // width
        return np.ascontiguousarray(
            w.reshape(DCH, 128, n, width).transpose(2, 1, 0, 3).reshape(
                n, 128, DCH * width).astype(dt))

    shared = {
        "wq": sbuf_layout(wq_flat, 128, fa),
        "wk": sbuf_layout(wk_flat, 128, fa),
        "wv": sbuf_layout(wv_aug, VQW, fa),
        "wo": sbuf_layout(np.ascontiguousarray(Wo), D, np.float16)[0],
        "bqk": bqk, "bv": bv_aug,
        "gamma": gamma.reshape(1, D).copy(),
        "beta": beta.reshape(1, D).copy(),
        "ones16": np.ones((1, 128), np.float16),
        "onesr": np.ones((1, 128), np.float32),
        "bo16": bo.reshape(1, D).astype(np.float16),
    }
    in_maps = []
    for b in range(B):
        mb = np.where(mask[b], 0.0, NEG_MASK).astype(np.float32)
        in_maps.append({
            **shared,
            "xt": np.ascontiguousarray(
                x[b].T.reshape(DCH, 128, S).transpose(1, 0, 2).reshape(
                    128, DCH * S).astype(fa)),
            "maskb": np.ascontiguousarray(mb.reshape(SBLK, 128).T),
        })
    return in_maps


def _get_program():
    global _PROGRAM
    if _PROGRAM is None:
        _PROGRAM = _build_program()
    return _PROGRAM


def kernel(**inputs):
    from concourse.bass_utils import run_bass_kernel_spmd

    nc = _get_program()
    in_maps = _host_inputs(inputs)
    res = run_bass_kernel_spmd(nc, in_maps, list(range(B)))
    return np.stack([res.results[b]["out"] for b in range(B)], axis=0)


if __name__ == "__main__":
    rng = np.random.default_rng(0)
    demo = {
        "input_tensor": rng.standard_normal((B, S, D)).astype(np.float32),
        "attention_mask": np.ones((B, S), bool),
        "Wq": (rng.standard_normal((H, D, DH)) * 0.03).astype(np.float32),
        "bq": (rng.standard_normal((H, DH)) * 0.03).astype(np.float32),
        "Wk": (rng.standard_normal((H, D, DH)) * 0.03).astype(np.float32),
        "bk": (rng.standard_normal((H, DH)) * 0.03).astype(np.float32),
        "Wv": (rng.standard_normal((H, D, DH)) * 0.03).astype(np.float32),
        "bv": (rng.standard_normal((H, DH)) * 0.03).astype(np.float32),
        "Wo": (rng.standard_normal((D, D)) * 0.03).astype(np.float32),
        "bo": (rng.standard_normal((D,)) * 0.03).astype(np.float32),
        "gamma": np.ones((D,), np.float32),
        "beta": np.zeros((D,), np.float32),
    }
    out = kernel(**demo)
    print("kernel ran, out shape", out.shape, "finite:", np.isfinite(out).all())


# revision 38
# speedup vs baseline: 1.0260x; 1.0143x over previous
"""Multi-head attention + layernorm Bass kernel for Trainium2, 8 cores.

Problem: B=8, S=1024, D=768, H=12 heads x DH=64, key-padding mask, softmax,
output projection, layernorm.  Sharding: pure data parallelism - one batch
element per NeuronCore, no collectives.

v2 design (ACT-exp is the throughput floor at ~110us; everything else must
hide under it):
  - fp8e4(+DoubleRow, K=256/matmul) for q/k/v projections and attn@V; these
    errors enter before the softmax average and wash out.  Scores and the
    output projection stay fp16.
  - weights prescaled x64 on host so fp8 stays in the normal range; the
    scale cancels through the softmax normalize (ctx*64 times 1/(64*den)),
    and for scores it folds into the exp scale 2^-15.
  - iblk-outer attention; out-projection blocks s0..3 interleave under the
    second iblk so only s4..7 are an exposed tail.
  - softmax denominators: ones-column trick in V; reciprocal_approx_fast on
    the psum row; DMA row-broadcast; in-place f16 multiply.
  - ~130 warmup matmuls during the input DMA so HAM reaches K=8/8 before
    real work; xt split across 3 DMA queues.
"""

import numpy as np

B, S, D, H, DH = 8, 1024, 768, 12, 64
NPAIR, NQUAD = H // 2, H // 4
SBLK = S // 128      # 8 key/row chunks
DCH = D // 128       # 6 contraction chunks
LN_EPS = 1e-5
NEG_MASK = -30.0
W64 = 64.0           # host weight prescale
EXP_SCALE = 1.0 / (64.0 * 64.0 * 8.0)   # qt64*kt64 -> scores/8
VW = 65              # per-head stride in V layout: [v64, ones]
VQW = 4 * VW         # 260, per-quad width
FP8 = False
N_WARM = 40

_PROGRAM = None


def _build_program():
    import concourse.bass as bass
    from concourse import bacc
    import concourse.tile as tile
    import concourse.mybir as mybir
    from contextlib import ExitStack

    F32 = mybir.dt.float32
    F16 = mybir.dt.float16
    F8 = mybir.dt.float8e4
    FA = F8 if FP8 else F16
    DR = mybir.MatmulPerfMode.DoubleRow if FP8 else None
    AF = mybir.ActivationFunctionType
    CP = 2 if FP8 else 1          # contraction chunks consumed per matmul

    nc = bacc.Bacc("TRN2", target_bir_lowering=False)

    xt_d = nc.dram_tensor("xt", [128, DCH * S], FA, kind="ExternalInput")
    wq_d = nc.dram_tensor("wq", [NPAIR, 128, DCH * 128], FA, kind="ExternalInput")
    wk_d = nc.dram_tensor("wk", [NPAIR, 128, DCH * 128], FA, kind="ExternalInput")
    wv_d = nc.dram_tensor("wv", [NQUAD, 128, DCH * VQW], FA, kind="ExternalInput")
    wo_d = nc.dram_tensor("wo", [128, DCH * D], F16, kind="ExternalInput")
    bqk_d = nc.dram_tensor("bqk", [128, 2 * NPAIR], F32, kind="ExternalInput")
    bv_d = nc.dram_tensor("bv", [1, NQUAD * VQW], F32, kind="ExternalInput")
    maskb_d = nc.dram_tensor("maskb", [128, SBLK], F32, kind="ExternalInput")
    gamma_d = nc.dram_tensor("gamma", [1, D], F32, kind="ExternalInput")
    beta_d = nc.dram_tensor("beta", [1, D], F32, kind="ExternalInput")
    ones_d = nc.dram_tensor("ones16", [1, 128], F16, kind="ExternalInput")
    onesr_d = nc.dram_tensor("onesr", [1, 128], mybir.dt.float32r,
                             kind="ExternalInput")
    bo_d = nc.dram_tensor("bo16", [1, D], F16, kind="ExternalInput")
    out_d = nc.dram_tensor("out", [S, D], F32, kind="ExternalOutput")

    # j -> (et group, slot in group); groups pair key-chunks for DoubleRow
    ET_SLOT = [(0, 0), (0, 1), (3, 0), (1, 0), (1, 1), (3, 1), (2, 0), (2, 1)]
    # group -> (v dim1 slice start, stop, step)
    GRP_V = {0: (0, 2, 1), 1: (3, 5, 1), 2: (6, 8, 1), 3: (2, 6, 3)}

    with tile.TileContext(nc) as tc, ExitStack() as ctx:
        const = ctx.enter_context(tc.tile_pool(name="const", bufs=1))
        xt_p = ctx.enter_context(tc.tile_pool(name="xt_p", bufs=1))
        w_p = ctx.enter_context(tc.tile_pool(name="w_p", bufs=1))
        qk_p = ctx.enter_context(tc.tile_pool(name="qk_p", bufs=1))
        v_p = ctx.enter_context(tc.tile_pool(name="v_p", bufs=1))
        e_p = ctx.enter_context(tc.tile_pool(name="e_p", bufs=1))
        cx_p = ctx.enter_context(tc.tile_pool(name="cx_p", bufs=1))
        z_p = ctx.enter_context(tc.tile_pool(name="z_p", bufs=1))
        ps = ctx.enter_context(tc.tile_pool(name="ps", bufs=1, space="PSUM"))

        # ---- warmup stationary (DVE memset, no DMA dependency) ----
        warm16 = const.tile([128, 64], F16)
        nc.vector.memset(warm16, 0.25)

        # ---- input DMAs, all on the sync queue: ordered so the first
        # projection (wq0/wk0 + xt) and first exp (bqk, maskb) unblock ASAP
        xt8 = xt_p.tile([128, DCH, S], FA, name="xt8")
        wq_ts = [w_p.tile([128, DCH, 128], FA, name="wqp", bufs=NPAIR)
                 for _ in range(NPAIR)]
        wk_ts = [w_p.tile([128, DCH, 128], FA, name="wkp", bufs=NPAIR)
                 for _ in range(NPAIR)]
        wv_ts = [w_p.tile([128, DCH, VQW], FA, name="wvq", bufs=NQUAD)
                 for _ in range(NQUAD)]
        bqk_t = const.tile([128, 2 * NPAIR], F32)
        mask_t = const.tile([128, SBLK], F32)
        bv_t = const.tile([128, NQUAD * VQW], F32)
        ones_t = const.tile([1, 128], F16)
        onesr_t = const.tile([1, 128], mybir.dt.float32r)
        bo_t = const.tile([1, D], F16)
        gamma_t = const.tile([128, D], F32)
        beta_t = const.tile([128, D], F32)
        woa = w_p.tile([128, DCH, D], F16, name="woa", bufs=1)

        nc.sync.dma_start(out=wq_ts[0], in_=wq_d[0])
        nc.sync.dma_start(out=wk_ts[0], in_=wk_d[0])
        nc.sync.dma_start(out=xt8[:, 0:2, :], in_=xt_d[:, 0:2 * S])
        nc.sync.dma_start(out=xt8[:, 2:4, :], in_=xt_d[:, 2 * S:4 * S])
        nc.sync.dma_start(out=xt8[:, 4:6, :], in_=xt_d[:, 4 * S:6 * S])
        nc.sync.dma_start(out=bqk_t, in_=bqk_d[:, :])
        nc.sync.dma_start(out=mask_t, in_=maskb_d[:, :])
        nc.sync.dma_start(out=wq_ts[1], in_=wq_d[1])
        nc.sync.dma_start(out=wk_ts[1], in_=wk_d[1])
        nc.sync.dma_start(out=ones_t, in_=ones_d[:, :])
        nc.sync.dma_start(out=onesr_t, in_=onesr_d[:, :])
        nc.sync.dma_start(out=bo_t, in_=bo_d[:, :])
        nc.sync.dma_start(out=wq_ts[2], in_=wq_d[2])
        nc.sync.dma_start(out=wk_ts[2], in_=wk_d[2])
        nc.sync.dma_start(out=wv_ts[0], in_=wv_d[0])
        nc.sync.dma_start(out=bv_t, in_=bv_d[0:1, :].to_broadcast([128, NQUAD * VQW]))
        for p in range(3, NPAIR):
            nc.sync.dma_start(out=wq_ts[p], in_=wq_d[p])
            nc.sync.dma_start(out=wk_ts[p], in_=wk_d[p])
        for q in range(1, NQUAD):
            nc.sync.dma_start(out=wv_ts[q], in_=wv_d[q])
        nc.sync.dma_start(out=woa, in_=wo_d[:, :])
        nc.sync.dma_start(out=gamma_t, in_=gamma_d[0:1, :].to_broadcast([128, D]))
        nc.sync.dma_start(out=beta_t, in_=beta_d[0:1, :].to_broadcast([128, D]))
        eps_t = const.tile([128, 1], F32)
        nc.vector.memset(eps_t, LN_EPS)
        magic_t = const.tile([128, 1], mybir.dt.int32)
        nc.vector.memset(magic_t, 0x5F3759DF)

        # ---- PE warmup: keep HAM busy during input DMA ----
        pw = ps.tile([64, 64], F32, name="pw", tag="pa", bufs=2,
                     padded_shape=[128, 512])
        for _ in range(N_WARM):
            nc.tensor.matmul(pw, warm16, warm16, start=True, stop=True)

        # ---- emit helpers ----
        v8 = [v_p.tile([128, SBLK, VQW], FA, name="v8", bufs=NQUAD)
              for _ in range(NQUAD)]
        qt = [qk_p.tile([128, S], F16, name="qt", bufs=NPAIR) for _ in range(NPAIR)]
        kt = [qk_p.tile([128, S], F16, name="kt", bufs=NPAIR) for _ in range(NPAIR)]
        ct = [cx_p.tile([128, S], F16, name="ct", bufs=NPAIR) for _ in range(NPAIR)]

        def emit_vquad(q):
            wv_t = wv_ts[q]

            def mk_v(s):
                def f():
                    psv = ps.tile([128, 512], F32, name="psv", tag="pa",
                                  bufs=2)
                    for ci in range(DCH // CP):
                        nc.tensor.matmul(
                            psv[:, 0:VQW],
                            xt8[:, CP * ci:CP * (ci + 1),
                                s * 128:(s + 1) * 128],
                            wv_t[:, CP * ci:CP * (ci + 1), :],
                            start=(ci == 0), stop=(ci == DCH // CP - 1),
                            perf_mode=DR)
                    with tc.high_priority(offset=350):
                        nc.vector.tensor_add(
                            out=v8[q][:, s, :], in0=psv[:, 0:VQW],
                            in1=bv_t[:, q * VQW:(q + 1) * VQW])
                return f

            for s in range(SBLK):
                pending.append(mk_v(s))

        def emit_proj(p):
            for dst, w_t, bcol in ((qt[p], wq_ts[p], p), (kt[p], wk_ts[p], NPAIR + p)):
                for half in range(2):
                    psq = ps.tile([128, 512], F32, name="psq", tag="pa", bufs=2)
                    for ci in range(DCH // CP):
                        nc.tensor.matmul(
                            psq,
                            w_t[:, CP * ci:CP * (ci + 1), :],
                            xt8[:, CP * ci:CP * (ci + 1),
                                half * 512:(half + 1) * 512],
                            start=(ci == 0), stop=(ci == DCH // CP - 1),
                            perf_mode=DR)
                    with tc.high_priority(offset=400):
                        nc.vector.tensor_scalar_add(
                            out=dst[:, half * 512:(half + 1) * 512], in0=psq,
                            scalar1=bqk_t[:, bcol:bcol + 1])

        ET = {}
        from collections import deque
        pending = deque()

        def fill(n):
            for _ in range(n):
                if pending:
                    pending.popleft()()

        def flush():
            while pending:
                pending.popleft()()

        def emit_scores(p, iblk):
            # scores + exp feed ACT; after each slot, drain two queued
            # dependency-free PE work units so the in-order PE never idles
            ets = []
            for j in range(SBLK):
                pst = ps.tile([128, 1024], F32, name="pst", tag="pb", bufs=2)
                nc.tensor.matmul(
                    pst[:, 0:512], kt[p][0:64, j * 128:(j + 1) * 128],
                    qt[p][0:64, iblk * 512:(iblk + 1) * 512],
                    start=True, stop=True, tile_position=(0, 0))
                nc.tensor.matmul(
                    pst[:, 512:1024], kt[p][64:128, j * 128:(j + 1) * 128],
                    qt[p][64:128, iblk * 512:(iblk + 1) * 512],
                    start=True, stop=True, tile_position=(64, 0))
                et = e_p.tile([128, 1024], FA, name="et", bufs=24)
                nc.scalar.activation(et, pst, AF.Exp,
                                     bias=mask_t[:, j:j + 1],
                                     scale=EXP_SCALE)
                ets.append(et)
                fill(2)
            ET[(p, iblk)] = ets

        def emit_ctx(p, iblk):
            qx = 2 * p // 4
            l0 = (2 * p) % 4
            ets = ET.pop((p, iblk))
            box = []

            def mk_ctx(j):
                def f():
                    if j == 0:
                        box.append(ps.tile([65, 1024], F32, name="pcx",
                                           tag="pc", bufs=1))
                    pcx = box[0]
                    for idx in range(2):
                        nc.tensor.matmul(
                            pcx[0:65, idx * 512:(idx + 1) * 512],
                            v8[qx][:, j,
                                   (l0 + idx) * VW:(l0 + idx + 1) * VW],
                            ets[j][:, idx * 512:(idx + 1) * 512],
                            start=(j == 0), stop=(j == SBLK - 1))
                return f

            def norm():
                pcx = box[0]
                rxs = z_p.tile([1, 1024], F32, name="rxs", bufs=3)
                rx = z_p.tile([1, 1024], F32, name="rx", bufs=3)
                with tc.high_priority(offset=300):
                    nc.vector.tensor_copy(out=rxs, in_=pcx[64:65, 0:1024])
                nc.vector.reciprocal_approx_fast(out=rx, in_=rxs)
                rx16 = z_p.tile([1, 1024], F16, name="rx16", bufs=3)
                nc.vector.tensor_copy(out=rx16, in_=rx)
                pbc = ps.tile([128, 512], F32, name="pbc", tag="pa", bufs=2)
                nc.tensor.matmul(pbc[0:64, :], ones_t[0:1, 0:64],
                                 rx16[0:1, 0:512], start=True, stop=True)
                nc.tensor.matmul(pbc[64:128, :], ones_t[0:1, 0:64],
                                 rx16[0:1, 512:1024], start=True, stop=True,
                                 tile_position=(0, 64))
                pb16 = z_p.tile([128, 512], F16, name="pb16", bufs=3)
                nc.vector.tensor_copy(out=pb16, in_=pbc)
                nc.vector.tensor_mul(
                    out=ct[p][0:64, iblk * 512:(iblk + 1) * 512],
                    in0=pcx[0:64, 0:512], in1=pb16[0:64, :])
                nc.vector.tensor_mul(
                    out=ct[p][64:128, iblk * 512:(iblk + 1) * 512],
                    in0=pcx[0:64, 512:1024], in1=pb16[64:128, :])

            for j in range(SBLK):
                pending.append(mk_ctx(j))
            pending.append(norm)

        def emit_out(s, direct=True):
            # alternate psum rings so consecutive out-blocks never wait on
            # each other's LN drain (depth-2 pipeline in the tail)
            ring = "pa" if s % 2 == 0 else "pb"
            box = {}

            def mk_half(key, d0, dn):
                def f():
                    pt = ps.tile([128, 512], F32, name="pso_" + key,
                                 tag=ring, bufs=2)
                    box[key] = pt
                    for c in range(NPAIR):
                        nc.tensor.matmul(
                            pt[:, 0:dn],
                            ct[c][:, s * 128:(s + 1) * 128],
                            woa[:, c, d0:d0 + dn],
                            start=(c == 0), stop=False)
                    nc.tensor.matmul(pt[:, 0:dn], ones_t,
                                     bo_t[0:1, d0:d0 + dn],
                                     start=False, stop=True)
                return f

            def ln():
                emit_ln(s, box["a"], box["b"])

            units = [mk_half("a", 0, 512), mk_half("b", 512, 256), ln]
            if direct:
                for u in units:
                    u()
            else:
                pending.extend(units)

        def emit_ln(s, pso_a, pso_b):
            stats = z_p.tile([128, 3, 6], F32, name="stats", bufs=2)
            with tc.high_priority(offset=600):
                nc.vector.bn_stats(out=stats[:, 0, :], in_=pso_a[:, 0:256])
                nc.vector.bn_stats(out=stats[:, 1, :], in_=pso_a[:, 256:512])
                nc.vector.bn_stats(out=stats[:, 2, :], in_=pso_b[:, 0:256])
                mv = z_p.tile([128, 2], F32, name="mv", bufs=2)
                nc.vector.bn_aggr(out=mv, in_=stats)
            # rstd = rsqrt(var+eps) via quake seed + 2 Newton steps, all on
            # DVE: keeps the ACT engine exp-only (no table-set thrash)
            I32 = mybir.dt.int32
            with tc.high_priority(offset=600):
                veps = z_p.tile([128, 1], F32, name="veps", bufs=2)
                nc.vector.tensor_scalar_add(out=veps, in0=mv[:, 1:2],
                                            scalar1=LN_EPS)
                hb = z_p.tile([128, 1], I32, name="hb", bufs=2)
                nc.vector.tensor_scalar(out=hb, in0=veps.bitcast(I32),
                                        scalar1=1, scalar2=None,
                                        op0=mybir.AluOpType.arith_shift_right)
                y0 = z_p.tile([128, 1], I32, name="y0", bufs=2)
                nc.vector.tensor_tensor(out=y0, in0=magic_t, in1=hb,
                                        op=mybir.AluOpType.subtract)
                rstd = y0.bitcast(F32)
                vm = z_p.tile([128, 1], F32, name="vm", bufs=2)
                nc.vector.tensor_scalar_mul(out=vm, in0=veps, scalar1=-0.5)
                tq = z_p.tile([128, 1], F32, name="tq", bufs=2)
                for _ in range(2):
                    # 2 Newton steps: rstd rel err ~5e-6
                    nc.vector.tensor_mul(out=tq, in0=rstd, in1=rstd)
                    nc.vector.tensor_scalar(out=tq, in0=tq, scalar1=vm,
                                            scalar2=1.5,
                                            op0=mybir.AluOpType.mult,
                                            op1=mybir.AluOpType.add)
                    nc.vector.tensor_mul(out=y0.bitcast(F32), in0=rstd, in1=tq)
                nmr = z_p.tile([128, 1], F32, name="nmr", bufs=2)
                nc.vector.tensor_scalar(out=nmr, in0=mv[:, 0:1], scalar1=rstd,
                                        scalar2=-1.0, op0=mybir.AluOpType.mult,
                                        op1=mybir.AluOpType.mult)
                z = z_p.tile([128, D], F32, name="z_sb", bufs=2)
                if s >= 4:
                    # tail: ACT is idle once the exps are done
                    nc.scalar.activation(z[:, 0:512], pso_a, AF.Identity,
                                         bias=nmr, scale=rstd)
                    nc.scalar.activation(z[:, 512:768], pso_b[:, 0:256],
                                         AF.Identity, bias=nmr, scale=rstd)
                else:
                    nc.vector.tensor_scalar(out=z[:, 0:512], in0=pso_a,
                                            scalar1=rstd, scalar2=nmr,
                                            op0=mybir.AluOpType.mult,
                                            op1=mybir.AluOpType.add)
                    nc.vector.tensor_scalar(out=z[:, 512:768],
                                            in0=pso_b[:, 0:256],
                                            scalar1=rstd, scalar2=nmr,
                                            op0=mybir.AluOpType.mult,
                                            op1=mybir.AluOpType.add)
            zf = z_p.tile([128, D], F32, name="zf", bufs=2)
            if s >= 6:
                # last blocks: keep the chain on DVE (gpsimd TT is 1.8us/op
                # and fully exposed at the end)
                nc.vector.tensor_mul(out=z, in0=z, in1=gamma_t)
                nc.vector.tensor_add(out=zf, in0=z, in1=beta_t)
            else:
                nc.gpsimd.tensor_mul(out=z, in0=z, in1=gamma_t)
                nc.gpsimd.tensor_add(out=zf, in0=z, in1=beta_t)
            nc.sync.dma_start(out=out_d[s * 128:(s + 1) * 128, :], in_=zf)

        # ---- braided emission: scores slots drain queued dep-free PE
        # units (ctx of earlier pairs, V blocks, out blocks) so the
        # in-order PE stream never idles on exp-paced dependencies ----
        emit_proj(0)
        emit_vquad(0)
        emit_scores(0, 0)
        emit_proj(1)
        emit_ctx(0, 0)
        emit_scores(1, 0)
        emit_proj(2)
        emit_vquad(1)
        emit_scores(2, 0)
        emit_proj(3)
        emit_ctx(1, 0)
        emit_scores(3, 0)
        emit_proj(4)
        emit_vquad(2)
        emit_ctx(2, 0)
        emit_scores(4, 0)
        emit_proj(5)
        emit_ctx(3, 0)
        emit_scores(5, 0)
        emit_ctx(4, 0)
        emit_scores(0, 1)
        emit_ctx(5, 0)
        emit_scores(1, 1)
        emit_ctx(0, 1)
        emit_scores(2, 1)
        emit_ctx(1, 1)
        emit_out(0, direct=False)
        emit_scores(3, 1)
        emit_ctx(2, 1)
        emit_out(1, direct=False)
        emit_scores(4, 1)
        emit_ctx(3, 1)
        emit_out(2, direct=False)
        emit_scores(5, 1)
        emit_out(3, direct=False)
        emit_ctx(4, 1)
        emit_ctx(5, 1)
        flush()
        for s in range(4, SBLK):
            emit_out(s)

    nc.compile()
    return nc


def _np_f8():
    import ml_dtypes
    return ml_dtypes.float8_e4m3fn


def _host_inputs(inputs):
    x = np.asarray(inputs["input_tensor"], np.float32)
    mask = np.asarray(inputs["attention_mask"])
    Wq = np.asarray(inputs["Wq"], np.float32)
    bq = np.asarray(inputs["bq"], np.float32)
    Wk = np.asarray(inputs["Wk"], np.float32)
    bk = np.asarray(inputs["bk"], np.float32)
    Wv = np.asarray(inputs["Wv"], np.float32)
    bv = np.asarray(inputs["bv"], np.float32)
    Wo = np.asarray(inputs["Wo"], np.float32)
    bo = np.asarray(inputs["bo"], np.float32)
    gamma = np.asarray(inputs["gamma"], np.float32)
    beta = np.asarray(inputs["beta"], np.float32)

    fa = _np_f8() if FP8 else np.float16

    wq_flat = np.ascontiguousarray(Wq.transpose(1, 0, 2).reshape(D, D)) * W64
    wk_flat = np.ascontiguousarray(Wk.transpose(1, 0, 2).reshape(D, D)) * W64
    bq_s = bq.reshape(D) * W64
    bk_s = bk.reshape(D) * W64

    # ones column FIRST per head: denominator lands at psum partition 0
    wv_aug = np.zeros((D, NQUAD * VQW), np.float32)
    bv_aug = np.zeros((1, NQUAD * VQW), np.float32)
    for h in range(H):
        q, l = divmod(h, 4)
        base = q * VQW + l * VW
        wv_aug[:, base:base + 64] = Wv[h] * W64
        bv_aug[0, base:base + 64] = bv[h] * W64
        bv_aug[0, base + 64] = W64

    bqk = np.zeros((128, 2 * NPAIR), np.float32)
    for p in range(NPAIR):
        bqk[:, p] = bq_s[p * 128:(p + 1) * 128]
        bqk[:, NPAIR + p] = bk_s[p * 128:(p + 1) * 128]

    def sbuf_layout(w, width, dt):
        # [D, n*width] -> [n, 128, DCH*width]: partition-major per tile
        n = w.shape[1] // width
        return np.ascontiguousarray(
            w.reshape(DCH, 128, n, width).transpose(2, 1, 0, 3).reshape(
                n, 128, DCH * width).astype(dt))

    shared = {
        "wq": sbuf_layout(wq_flat, 128, fa),
        "wk": sbuf_layout(wk_flat, 128, fa),
        "wv": sbuf_layout(wv_aug, VQW, fa),
        "wo": sbuf_layout(np.ascontiguousarray(Wo), D, np.float16)[0],
        "bqk": bqk, "bv": bv_aug,
        "gamma": gamma.reshape(1, D).copy(),
        "beta": beta.reshape(1, D).copy(),
        "ones16": np.ones((1, 128), np.float16),
        "onesr": np.ones((1, 128), np.float32),
        "bo16": bo.reshape(1, D).astype(np.float16),
    }
    in_maps = []
    for b in range(B):
        mb = np.where(mask[b], 0.0, NEG_MASK).astype(np.float32)
        in_maps.append({
            **shared,
            "xt": np.ascontiguousarray(
                x[b].T.reshape(DCH, 128, S).transpose(1, 0, 2).reshape(
                    128, DCH * S).astype(fa)),
            "maskb": np.ascontiguousarray(mb.reshape(SBLK, 128).T),
        })
    return in_maps


def _get_program():
    global _PROGRAM
    if _PROGRAM is None:
        _PROGRAM = _build_program()
    return _PROGRAM


def kernel(**inputs):
    from concourse.bass_utils import run_bass_kernel_spmd

    nc = _get_program()
    in_maps = _host_inputs(inputs)
    res = run_bass_kernel_spmd(nc, in_maps, list(range(B)))
    return np.stack([res.results[b]["out"] for b in range(B)], axis=0)


if __name__ == "__main__":
    rng = np.random.default_rng(0)
    demo = {
        "input_tensor": rng.standard_normal((B, S, D)).astype(np.float32),
        "attention_mask": np.ones((B, S), bool),
        "Wq": (rng.standard_normal((H, D, DH)) * 0.03).astype(np.float32),
        "bq": (rng.standard_normal((H, DH)) * 0.03).astype(np.float32),
        "Wk": (rng.standard_normal((H, D, DH)) * 0.03).astype(np.float32),
        "bk": (rng.standard_normal((H, DH)) * 0.03).astype(np.float32),
        "Wv": (rng.standard_normal((H, D, DH)) * 0.03).astype(np.float32),
        "bv": (rng.standard_normal((H, DH)) * 0.03).astype(np.float32),
        "Wo": (rng.standard_normal((D, D)) * 0.03).astype(np.float32),
        "bo": (rng.standard_normal((D,)) * 0.03).astype(np.float32),
        "gamma": np.ones((D,), np.float32),
        "beta": np.zeros((D,), np.float32),
    }
    out = kernel(**demo)
    print("kernel ran, out shape", out.shape, "finite:", np.isfinite(out).all())
